# revision 26
# baseline (speedup 1.0000x reference)
"""CTRNN kernel for Trainium2 (Bass/Tile), data-parallel over 8 NeuronCores.

Reference semantics (TAU=1.0 so alpha=1, u carries nothing):
    drive = x @ I_w.T + v                # [B, H], constant over time
    per step: u = drive + z @ H_w.T ; z = tanh(u) ; y = sigmoid(z @ O_w.T + m)

Layout strategy (per core, B=1024):
  - state kept transposed: zT[k-tile][128, B] (h on partitions, batch on free)
  - H_w.T tiles are the stationary matmul operand, zT streams in float32r
    (1 cycle/row at N>=256, ~1.7e-4 rel err/matmul vs 2.5e-3 bf16; the
    recurrence amplifies per-step noise ~10x over 256 steps on weakly-driven
    batch elements, so bf16 is not accurate enough)
  - u accumulated in PSUM fp32; drive added on DVE; tanh on ACT writes f32r z
  - readout logits land in PSUM partition 0 (f32r matmuls cannot write other
    partitions); DVE packs them at 32-aligned strips, sigmoid per 4 steps
  - per 4-step chunk, each [128, 128] b-tile is PE-transposed to [b, (t,o)]
    layout, DVE-compacted (dropping the 20-row strip gaps), and DMA'd out
"""

import os
import sys

for _p in ("/opt/trn_rl_repo", "/root/.axon_site/_ro/trn_rl_repo"):
    if os.path.isdir(_p) and _p not in sys.path:
        sys.path.insert(0, _p)

import ml_dtypes
import numpy as np

N_CORES = 8
B_TOTAL = 8192
B = B_TOTAL // N_CORES  # 1024
H = 512
O = 12
T_STEPS = 256
P = 128
KT = H // P  # 4 k/h' tiles
NB = 512  # moving free-dim per matmul (one PSUM bank of fp32)
BC = B // NB  # 2 batch chunks
CH = 4  # steps per output chunk: step r's readout lands at psum partition 32*r

_BUILT = {}
LAST_RESULTS = None


def _build(t_steps):
    import concourse.mybir as mybir
    import concourse.tile as tile
    from concourse import bacc
    from concourse.masks import make_identity

    f32 = mybir.dt.float32
    f32r = mybir.dt.float32r
    AF = mybir.ActivationFunctionType

    nc = bacc.Bacc(
        "TRN2",
        target_bir_lowering=False,
        debug=False,
        enable_asserts=False,
        num_devices=N_CORES,
    )

    drv_d = nc.dram_tensor("drivet", [H, B], f32, kind="ExternalInput")
    hw_d = nc.dram_tensor("hwt", [H, H], f32r, kind="ExternalInput")
    ow_d = nc.dram_tensor("owt", [H, O], f32r, kind="ExternalInput")
    mb_d = nc.dram_tensor("mb", [P, 1], f32, kind="ExternalInput")
    y_d = nc.dram_tensor("y", [B, t_steps * O], f32, kind="ExternalOutput")

    with tile.TileContext(nc) as tc:
        with (
            tc.tile_pool(name="const", bufs=1) as cpool,
            tc.tile_pool(name="state", bufs=1) as spool,
            tc.tile_pool(name="usb", bufs=4) as upool,
            tc.tile_pool(name="ych", bufs=2) as ypool,
            tc.tile_pool(name="yst", bufs=3) as ystage,
            tc.tile_pool(name="pu", bufs=4, space="PSUM") as pu,
            tc.tile_pool(name="py", bufs=2, space="PSUM") as py,
            tc.tile_pool(name="ptr", bufs=2, space="PSUM") as ptr,
        ):
            # ---- constants ----
            hsb = [
                [
                    cpool.tile([P, P], f32r, name=f"hw_{k}_{h}", tag=f"hw_{k}_{h}")
                    for h in range(KT)
                ]
                for k in range(KT)
            ]
            for k in range(KT):
                for h in range(KT):
                    nc.sync.dma_start(
                        hsb[k][h][:],
                        hw_d[k * P : (k + 1) * P, h * P : (h + 1) * P],
                    )
            osb = [cpool.tile([P, O], f32r, name=f"ow_{k}", tag=f"ow_{k}") for k in range(KT)]
            for k in range(KT):
                nc.sync.dma_start(osb[k][:], ow_d[k * P : (k + 1) * P, :])
            drv = [cpool.tile([P, B], f32, name=f"drv_{h}", tag=f"drv_{h}") for h in range(KT)]
            for h in range(KT):
                nc.sync.dma_start(drv[h][:], drv_d[h * P : (h + 1) * P, :])
            mb = cpool.tile([P, 1], f32, name="mb_sb", tag="mb_sb")
            nc.sync.dma_start(mb[:], mb_d[:, :])
            ident = cpool.tile([P, P], f32, name="ident", tag="ident")
            make_identity(nc, ident[:])

            # ---- state (ping-pong by step parity) ----
            zs = [
                [
                    spool.tile([P, B], f32r, name=f"z{pp}_{k}", tag=f"z{pp}_{k}")
                    for k in range(KT)
                ]
                for pp in range(2)
            ]
            # z0 = 0, via tanh(0) since fp32r consumers need a rounding producer
            zinit = upool.tile([P, B], f32, name="zinit", tag="zinit", bufs=1)
            nc.any.memset(zinit[:], 0.0)
            for k in range(KT):
                nc.scalar.activation(zs[0][k][:], zinit[:], AF.Tanh)

            ychunk = None
            for t in range(t_steps):
                zc = zs[t % 2]
                zn = zs[(t + 1) % 2]
                r = t % CH
                if r == 0:
                    # logits for 4 steps packed at 32-aligned partition strips
                    ychunk = ypool.tile([P, B], f32, name="ychunk", tag="ychunk")

                # ---- two independent batch streams, interleaved so one
                # stream's MM burst hides the other's add+tanh tail ----
                for s in range(BC):
                    sl = slice(s * NB, (s + 1) * NB)
                    for h in range(KT):
                        ups = pu.tile([P, NB], f32, name="ups", tag="ups")
                        for k in range(KT):
                            nc.tensor.matmul(
                                ups[:, :],
                                lhsT=hsb[k][h][:],
                                rhs=zc[k][:, sl],
                                start=(k == 0),
                                stop=(k == KT - 1),
                            )
                        ut = upool.tile([P, NB], f32, name="ut", tag="ut")
                        nc.vector.tensor_add(ut[:], ups[:], drv[h][:, sl])
                        nc.scalar.activation(zn[h][:, sl], ut[:], AF.Tanh)
                    yps = py.tile([O, NB], f32, name="yps", tag="yps")
                    for k in range(KT):
                        nc.tensor.matmul(
                            yps[:, :],
                            lhsT=osb[k][:],
                            rhs=zn[k][:, sl],
                            start=(k == 0),
                            stop=(k == KT - 1),
                        )
                    nc.vector.tensor_copy(ychunk[32 * r : 32 * r + O, sl], yps[:])

                # ---- chunk flush: sigmoid, transpose to [b, (t, o)], DMA out ----
                if r == CH - 1:
                    t0 = t - CH + 1
                    nc.scalar.activation(ychunk[:], ychunk[:], AF.Sigmoid, bias=mb[:])
                    for bt in range(B // P):
                        trp = ptr.tile([P, P], f32, name="trp", tag="trp")
                        nc.tensor.transpose(
                            trp[:],
                            ychunk[:, bt * P : (bt + 1) * P],
                            ident[:],
                        )
                        yout = ystage.tile([P, CH * O], f32, name="yout", tag="yout")
                        nc.vector.tensor_copy(
                            yout[:],
                            trp.rearrange("p (r g) -> p r g", g=32)[:, :, 0:O],
                        )
                        nc.sync.dma_start(
                            y_d[bt * P : (bt + 1) * P, t0 * O : (t0 + CH) * O],
                            yout[:],
                        )
    nc.compile()
    return nc


def _get_nc(t_steps=T_STEPS):
    if t_steps not in _BUILT:
        _BUILT[t_steps] = _build(t_steps)
    return _BUILT[t_steps]


def _prep_in_maps(x, I_w, H_w, O_w, v, m):
    x = np.asarray(x, np.float32)
    I_w = np.asarray(I_w, np.float32)
    H_w = np.asarray(H_w, np.float32)
    O_w = np.asarray(O_w, np.float32)
    v = np.asarray(v, np.float32)
    m = np.asarray(m, np.float32)

    hwt = np.ascontiguousarray(H_w.T.astype(np.float32))
    owt = np.ascontiguousarray(O_w.T.astype(np.float32))
    # m bias replicated at the 32-aligned col-strip offsets used by the
    # packed readout (step r of each 4-step chunk sits at partitions 32r).
    mbcol = np.zeros((P, 1), np.float32)
    for r in range(CH):
        mbcol[32 * r : 32 * r + O, 0] = m

    in_maps = []
    for c in range(N_CORES):
        xc = x[c * B : (c + 1) * B]  # [B, 1]
        drive = xc @ I_w.T + v  # [B, H] fp32
        drivet = np.ascontiguousarray(drive.T.astype(np.float32))  # [H, B]
        in_maps.append({"drivet": drivet, "hwt": hwt, "owt": owt, "mb": mbcol})
    return in_maps


def kernel(x, T, I_w, H_w, O_w, v, m, _t_steps=None, _trace=False):
    global LAST_RESULTS
    from concourse.bass_utils import run_bass_kernel_spmd

    t_steps = int(_t_steps if _t_steps is not None else T)
    nc = _get_nc(t_steps)

    if _trace:
        # NTFF tracing under axon needs the antenv.axon_hooks profile hook;
        # fall back to untraced execution when it's not available.
        try:
            from antenv.axon_hooks import get_axon_ntff_profile_hook

            _trace = get_axon_ntff_profile_hook() is not None
        except Exception:
            _trace = False

    in_maps = _prep_in_maps(x, I_w, H_w, O_w, v, m)
    res = run_bass_kernel_spmd(
        nc, in_maps, core_ids=list(range(N_CORES)), trace=_trace
    )
    LAST_RESULTS = res
    out = np.concatenate(
        [r["y"].reshape(B, t_steps, O) for r in res.results], axis=0
    )
    return out


def bench(x, T, I_w, H_w, O_w, v, m, _t_steps=None, n_iters=5, repeats=1):
    """Time device execution with device-resident inputs (ns, min over iters).

    Replicates bass2jax.run_bass_via_pjrt's shard_map plumbing so the
    repeated timed calls exclude host<->device transfer of inputs/outputs.
    With repeats=R the NEFF is executed R times per dispatch, serialized by
    threading the output buffer through each call — the (R2-R1) slope then
    isolates pure device execution from the axon dispatch floor.
    """
    import jax
    from jax.sharding import Mesh, NamedSharding, PartitionSpec
    from jax.experimental.shard_map import shard_map

    import concourse.mybir as mybir
    from concourse.bass2jax import (
        _bass_exec_p,
        install_neuronx_cc_hook,
        partition_id_tensor,
    )

    t_steps = int(_t_steps if _t_steps is not None else T)
    nc = _get_nc(t_steps)
    install_neuronx_cc_hook()
    in_maps = _prep_in_maps(x, I_w, H_w, O_w, v, m)

    partition_name = (
        nc.partition_id_tensor.name if nc.partition_id_tensor else None
    )
    in_names, out_names, out_avals, zero_outs = [], [], [], []
    for alloc in nc.m.functions[0].allocations:
        if not isinstance(alloc, mybir.MemoryLocationSet):
            continue
        name = alloc.memorylocations[0].name
        if alloc.kind == "ExternalInput":
            if name != partition_name:
                in_names.append(name)
        elif alloc.kind == "ExternalOutput":
            shape = tuple(alloc.tensor_shape)
            dtype = mybir.dt.np(alloc.dtype)
            out_names.append(name)
            out_avals.append(jax.core.ShapedArray(shape, dtype))
            zero_outs.append(np.zeros(shape, dtype))
    n_params = len(in_names)
    in_names = in_names + out_names
    if partition_name is not None:
        in_names.append(partition_name)

    def _body(*args):
        ins = list(args[:n_params])
        outs = list(args[n_params:])
        for _ in range(repeats):
            operands = ins + outs
            if partition_name is not None:
                operands.append(partition_id_tensor())
            outs = list(
                _bass_exec_p.bind(
                    *operands,
                    out_avals=tuple(out_avals),
                    in_names=tuple(in_names),
                    out_names=tuple(out_names),
                    lowering_input_output_aliases=(),
                    sim_require_finite=True,
                    sim_require_nnan=True,
                    nc=nc,
                )
            )
        return tuple(outs)

    devices = jax.devices()[:N_CORES]
    mesh = Mesh(np.asarray(devices), ("core",))
    n_outs = len(out_names)
    donate = tuple(range(n_params, n_params + n_outs))
    sharded = jax.jit(
        shard_map(
            _body,
            mesh=mesh,
            in_specs=(PartitionSpec("core"),) * (n_params + n_outs),
            out_specs=(PartitionSpec("core"),) * n_outs,
            check_rep=False,
        ),
        donate_argnums=donate,
        keep_unused=True,
    )
    sh = NamedSharding(mesh, PartitionSpec("core"))
    concat_in = [
        np.concatenate([np.asarray(in_maps[c][in_names[i]]) for c in range(N_CORES)], axis=0)
        for i in range(n_params)
    ]
    dev_in = [jax.device_put(a, sh) for a in concat_in]
    big_zeros = [np.zeros((N_CORES * z.shape[0], *z.shape[1:]), z.dtype) for z in zero_outs]

    import time as _time

    times = []
    out = None
    for it in range(n_iters + 1):  # first call = compile/warmup, excluded
        dev_zeros = [jax.device_put(z, sh) for z in big_zeros]
        jax.block_until_ready(dev_zeros)
        t0 = _time.perf_counter()
        out = sharded(*dev_in, *dev_zeros)
        jax.block_until_ready(out)
        dt = _time.perf_counter() - t0
        if it > 0:
            times.append(dt)
    result = np.asarray(out[0]).reshape(N_CORES, B, t_steps, O).reshape(B_TOTAL, t_steps, O)
    return int(min(times) * 1e9), times, result

